# revision 21
# baseline (speedup 1.0000x reference)
"""Trainium2 Bass kernel for nn_AttBiMambaModel (bidirectional Mamba encoder).

Sharding: data-parallel over batch (16 -> 2 per core, 8 cores, no
collectives).  On-device layout is [channel-on-partitions, time-on-free]
throughout, so every projection is a natural PE matmul.

The selective scan runs as one DVE tensor_tensor_scan per state index n over
[128, G*L] (the DI partition-groups are chained along the free axis; the
recurrence is reset at each block start by zeroing dA[:, block_start], which
is exact because h_{-1} = 0).  dA = exp(dt * A[:, n]) is a single ScalarE Exp
with constant immediate scale a_n = A[0, n]; A is d-independent by
construction in the reference's _init_params (asserted on host).
"""
import numpy as np
import ml_dtypes

import concourse.bacc as bacc
import concourse.tile as tile
from concourse import mybir
from concourse.bass_utils import run_bass_kernel_spmd

BF16 = ml_dtypes.bfloat16

FULL = dict(B=16, L=1024, D=512, DI=1024, N=64, RANK=32, DCONV=4, NL=2, DW=300,
            NCORES=8)

_nc_cache = {}


def build_nc(cfg):
    """Per-core Bass program; see _prep for the input-map layout."""
    B, L, D, DI, N = cfg["BPC"], cfg["L"], cfg["D"], cfg["DI"], cfg["N"]
    RANK, DCONV, NL, DW = cfg["RANK"], cfg["DCONV"], cfg["NL"], cfg["DW"]
    DG = D // 128          # partition groups of model dim
    G = DI // 128          # partition groups of inner dim
    GL = G * L
    LP = L + DCONV - 1     # padded time for conv input
    f32 = mybir.dt.float32
    bf16 = mybir.dt.bfloat16
    FMAX = min(L, 512)     # fp32 psum free limit per matmul
    an = cfg["an"]
    score_const = cfg["score_const"]
    sim_safe = cfg.get("sim_safe", False)
    prec = cfg.get("prec", set())

    def pdt(name):
        return mybir.dt.float32 if name in prec else mybir.dt.bfloat16

    nc = bacc.Bacc("TRN2", target_bir_lowering=False, num_devices=cfg["NCORES"])

    def din(name, shape, dtype=bf16):
        return nc.dram_tensor(name, list(shape), dtype, kind="ExternalInput").ap()

    xT = din("xT", [B, DW, L], f32)
    mask_in = din("mask", [B, L])
    in_w = din("in_w", [DW, D], f32)
    in_b = din("in_b", [128, DG], f32)
    lng = din("lng", [128, DG * NL], f32)
    lnb = din("lnb", [128, DG * NL], f32)
    w2col = din("w2col", [D, 1], f32)
    pjw = din("pjw", [D, 2], f32)
    pjb = din("pjb", [1, 2], f32)
    P = {}
    for l in range(NL):
        for r in range(2):
            s = f"{l}{r}"
            P[s] = dict(
                ipw=din(f"ipw_{s}", [D, 2 * DI]),
                cw=din(f"cw_{s}", [128, G * DCONV], f32),
                cb=din(f"cb_{s}", [128, G], f32),
                xpw=din(f"xpw_{s}", [DI, RANK + 2 * N]),
                dtw=din(f"dtw_{s}", [RANK, DI]),
                dtb=din(f"dtb_{s}", [128, G], f32),
                dv=din(f"dv_{s}", [128, G], f32),
                opw=din(f"opw_{s}", [DI, D]),
            )
    out_t = nc.dram_tensor("out", [B, 2], f32, kind="ExternalOutput").ap()
    debug = cfg.get("debug", False)
    if debug:
        def tap(nm, tiles):
            if not isinstance(tiles, list):
                tiles = [tiles]
            for g, t in enumerate(tiles):
                width = t.shape[1]
                dt_ = t.dtype
                dten = nc.dram_tensor(f"dbg_{nm}_{g}", [128, width], dt_,
                                      kind="ExternalOutput").ap()
                nc.sync.dma_start(out=dten, in_=t[:, :])
    z_spill = {f"{l}{r}": nc.dram_tensor(f"z_spill_{l}{r}", [B, 128, GL], bf16,
                                         kind="Internal").ap()
               for l in range(NL) for r in range(2)}
    bc_spill = {f"{l}{r}": nc.dram_tensor(f"bc_spill_{l}{r}", [B, 2 * N, L],
                                          pdt("BC"), kind="Internal").ap()
                for l in range(NL) for r in range(2)}

    AF = mybir.ActivationFunctionType
    OP = mybir.AluOpType

    with tile.TileContext(nc) as tc:
        import contextlib
        ctx = contextlib.ExitStack()
        with ctx:
            consts = ctx.enter_context(tc.tile_pool(name="consts", bufs=1))
            wpool = ctx.enter_context(tc.tile_pool(name="wpool", bufs=1))
            big = ctx.enter_context(tc.tile_pool(name="big", bufs=1))
            work = ctx.enter_context(tc.tile_pool(name="work", bufs=4))
            small = ctx.enter_context(tc.tile_pool(name="small", bufs=4))
            psA = ctx.enter_context(tc.tile_pool(name="psA", bufs=3, space="PSUM"))
            psB = ctx.enter_context(tc.tile_pool(name="psB", bufs=1, space="PSUM"))

            # ---- constants ----
            ones_col_f32 = consts.tile([128, 1], f32, tag="c1")
            nc.vector.memset(ones_col_f32, 1.0)
            ones_col_bf = consts.tile([128, 1], bf16, tag="c2")
            nc.vector.memset(ones_col_bf, 1.0)
            ones_row_bf = consts.tile([1, 128], bf16, tag="c3")
            nc.vector.memset(ones_row_bf, 1.0)
            ones_row_f32 = consts.tile([1, 128], f32, tag="c4")
            nc.vector.memset(ones_row_f32, 1.0)

            sb_inb = consts.tile([128, DG], f32, tag="c5")
            nc.sync.dma_start(out=sb_inb, in_=in_b)
            sb_lng = consts.tile([128, DG * NL], f32, tag="c6")
            nc.sync.dma_start(out=sb_lng, in_=lng)
            sb_lnb = consts.tile([128, DG * NL], f32, tag="c7")
            nc.sync.dma_start(out=sb_lnb, in_=lnb)
            dw_chunks = []
            o = 0
            while o < DW:
                c = min(128, DW - o)
                dw_chunks.append((o, c))
                o += c
            sb_w2 = consts.tile([128, DG], f32, tag="c8")
            nc.sync.dma_start(out=sb_w2.unsqueeze(2),
                              in_=w2col.rearrange("(g p) one -> p g one", p=128))
            sb_pjw = consts.tile([128, DG * 2], f32, tag="c9")
            nc.sync.dma_start(
                out=sb_pjw[:, :].rearrange("p (g two) -> p g two", g=DG),
                in_=pjw.rearrange("(g p) two -> p g two", p=128))
            sb_pjb = consts.tile([1, 2], f32, tag="c10")
            nc.sync.dma_start(out=sb_pjb, in_=pjb)
            def emit_silu(out_ap, in_ap, bias=0.0):
                if not sim_safe:
                    nc.scalar.activation(out=out_ap, in_=in_ap, func=AF.Silu,
                                         bias=bias, scale=1.0)
                else:
                    shp = [in_ap.shape[0], int(np.prod(in_ap.shape[1:]))]
                    tv = work.tile(shp, f32, tag="silv", name="silv")
                    nc.scalar.activation(out=tv, in_=in_ap, func=AF.Identity,
                                         bias=bias, scale=1.0)
                    tsg = work.tile(shp, f32, tag="silsg", name="silsg")
                    nc.scalar.activation(out=tsg, in_=in_ap, func=AF.Sigmoid,
                                         bias=bias, scale=1.0)
                    nc.vector.tensor_tensor(out=out_ap, in0=tv[:, :],
                                            in1=tsg[:, :], op=OP.mult)

            eps_t = consts.tile([1, 1], f32, tag="c11")
            nc.vector.memset(eps_t, 1e-5)
            neg1e9 = consts.tile([1, 1], f32, tag="c12")
            nc.vector.memset(neg1e9, -1.0e9)

            for b in range(B):
                # ======= per-batch persistent state =======
                h_res = [big.tile([128, L], f32, tag=f"h_{g}", name=f"h_{g}")
                         for g in range(DG)]
                mask_rep = big.tile([128, L], bf16, tag="mrep")
                mask_row = small.tile([1, L], bf16, tag="mrow", bufs=1)

                # ======= embedding: h = (E @ in_w + b) * mask =======
                nc.sync.dma_start(out=mask_row, in_=mask_in[b:b + 1, :])
                ps = psB.tile([128, L], f32, tag="psB")
                for fo in range(0, L, FMAX):
                    nc.tensor.matmul(out=ps[:, fo:fo + FMAX],
                                     lhsT=ones_row_bf[:, :],
                                     rhs=mask_row[:, fo:fo + FMAX],
                                     start=True, stop=True)
                nc.scalar.copy(out=mask_rep[:, :], in_=ps[:, :])

                inw_sb, xt_sb = [], []
                for ki, (o, c) in enumerate(dw_chunks):
                    wt = wpool.tile([128, D], f32, tag=f"wip{ki}",
                                    name=f"inw{ki}")
                    nc.sync.dma_start(out=wt[:c, :], in_=in_w[o:o + c, :])
                    inw_sb.append(wt)
                    t = work.tile([128, L], f32, tag="scr4", name=f"xtin{ki}")
                    nc.sync.dma_start(out=t[:c, :], in_=xT[b, o:o + c, :])
                    xt_sb.append(t)
                for g in range(DG):
                    ps = psA.tile([128, L], f32, tag="psA")
                    for fo in range(0, L, FMAX):
                        for ki, (o, c) in enumerate(dw_chunks):
                            nc.tensor.matmul(
                                out=ps[:, fo:fo + FMAX],
                                lhsT=inw_sb[ki][:c, g * 128:(g + 1) * 128],
                                rhs=xt_sb[ki][:c, fo:fo + FMAX],
                                start=(ki == 0), stop=(ki == len(dw_chunks) - 1))
                    tmp = work.tile([128, L], f32, tag="scr4", name="embtmp")
                    nc.scalar.activation(out=tmp, in_=ps[:, :], func=AF.Identity,
                                         bias=sb_inb[:, g:g + 1], scale=1.0)
                    nc.vector.tensor_tensor(out=h_res[g][:, :], in0=tmp[:, :],
                                            in1=mask_rep[:, :], op=OP.mult)
                if debug and b == 0:
                    tap("h_emb", h_res)

                # ======= layers =======
                for l in range(NL):
                    # ---- layernorm -> x (bf16) ----
                    x_ln = [big.tile([128, L], pdt("x"), tag=f"x_{g}", name=f"x_{g}")
                            for g in range(DG)]
                    ps_mu = psA.tile([128, L], f32, tag="psA")
                    for fo in range(0, L, FMAX):
                        for g in range(DG):
                            nc.tensor.matmul(out=ps_mu[0:1, fo:fo + FMAX],
                                             lhsT=ones_col_f32[:, :],
                                             rhs=h_res[g][:, fo:fo + FMAX],
                                             start=(g == 0), stop=(g == DG - 1))
                    hsq = work.tile([128, L], f32, tag="scr4", name="hsq")
                    ps_ss = psA.tile([128, L], f32, tag="psA")
                    for g in range(DG):
                        nc.scalar.square(out=hsq[:, :], in_=h_res[g][:, :])
                        for fo in range(0, L, FMAX):
                            nc.tensor.matmul(out=ps_ss[0:1, fo:fo + FMAX],
                                             lhsT=ones_col_f32[:, :],
                                             rhs=hsq[:, fo:fo + FMAX],
                                             start=(g == 0), stop=(g == DG - 1))
                    mu = small.tile([1, L], f32, tag="row4", name="mu")
                    nc.scalar.mul(out=mu, in_=ps_mu[0:1, :], mul=1.0 / D)
                    musq = small.tile([1, L], f32, tag="row4", name="musq")
                    nc.vector.tensor_tensor(out=musq, in0=mu[:, :], in1=mu[:, :],
                                            op=OP.mult)
                    var = small.tile([1, L], f32, tag="row4", name="var")
                    nc.vector.scalar_tensor_tensor(out=var, in0=ps_ss[0:1, :],
                                                   scalar=1.0 / D, in1=musq[:, :],
                                                   op0=OP.mult, op1=OP.subtract)
                    rstd = small.tile([1, L], f32, tag="row4", name="rstd")
                    nc.scalar.activation(out=rstd, in_=var[:, :], func=AF.Sqrt,
                                         bias=eps_t[:, :], scale=1.0)
                    nc.vector.reciprocal(out=rstd[:, :], in_=rstd[:, :])
                    ps_mur = psA.tile([128, L], f32, tag="psA")
                    ps_rsr = psB.tile([128, L], f32, tag="psB")
                    for fo in range(0, L, FMAX):
                        nc.tensor.matmul(out=ps_mur[:, fo:fo + FMAX],
                                         lhsT=ones_row_f32[:, :],
                                         rhs=mu[:, fo:fo + FMAX],
                                         start=True, stop=True)
                        nc.tensor.matmul(out=ps_rsr[:, fo:fo + FMAX],
                                         lhsT=ones_row_f32[:, :],
                                         rhs=rstd[:, fo:fo + FMAX],
                                         start=True, stop=True)
                    for g in range(DG):
                        t1 = work.tile([128, L], f32, tag="scr4", name="lnt1")
                        nc.vector.tensor_tensor(out=t1, in0=h_res[g][:, :],
                                                in1=ps_mur[:, :], op=OP.subtract)
                        t2 = work.tile([128, L], f32, tag="scr4", name="lnt2")
                        nc.vector.tensor_tensor(out=t2, in0=t1[:, :],
                                                in1=ps_rsr[:, :], op=OP.mult)
                        col = l * DG + g
                        nc.vector.tensor_scalar(
                            out=x_ln[g][:, :], in0=t2[:, :],
                            scalar1=sb_lng[:, col:col + 1],
                            scalar2=sb_lnb[:, col:col + 1],
                            op0=OP.mult, op1=OP.add)

                    if debug and b == 0 and l == 0:
                        tap("x_ln0", x_ln)

                    # ---- mamba, each direction ----
                    for r in range(2):
                        s = f"{l}{r}"
                        pp = P[s]
                        w_ip = [wpool.tile([128, DI], bf16, tag=f"wip{k}",
                                           name=f"wipa{k}") for k in range(DG)]
                        for k in range(DG):
                            nc.sync.dma_start(
                                out=w_ip[k],
                                in_=pp["ipw"][k * 128:(k + 1) * 128, 0:DI])
                        w_xp = [wpool.tile([128, RANK + 2 * N], bf16,
                                           tag=f"wxp{k}", name=f"wxp{k}")
                                for k in range(G)]
                        for k in range(G):
                            nc.sync.dma_start(
                                out=w_xp[k], in_=pp["xpw"][k * 128:(k + 1) * 128, :])
                        w_dt = wpool.tile([RANK, DI], bf16, tag="wdt")
                        nc.sync.dma_start(out=w_dt, in_=pp["dtw"])
                        w_op = [wpool.tile([128, D], bf16, tag=f"wop{k}", name=f"wop{k}")
                                for k in range(G)]
                        for k in range(G):
                            nc.sync.dma_start(
                                out=w_op[k], in_=pp["opw"][k * 128:(k + 1) * 128, :])
                        w_cw = wpool.tile([128, G * DCONV], f32, tag="wcw")
                        nc.sync.dma_start(out=w_cw, in_=pp["cw"])
                        w_cb = wpool.tile([128, G], f32, tag="wcb")
                        nc.sync.dma_start(out=w_cb, in_=pp["cb"])
                        w_dtb = wpool.tile([128, G], f32, tag="wdtb")
                        nc.sync.dma_start(out=w_dtb, in_=pp["dtb"])
                        w_dv = wpool.tile([128, G], f32, tag="wdv")
                        nc.sync.dma_start(out=w_dv, in_=pp["dv"])

                        def rr(ap):  # time-reverse for the bwd direction
                            return ap[:, ::-1] if r == 1 else ap

                        # ---- in_proj -> xc (padded), z (spilled) ----
                        xc_pad = big.tile([128, G * LP], pdt("xc"), tag="xcpad")
                        xcp = xc_pad[:, :].rearrange("p (g t) -> p g t", g=G)
                        nc.vector.memset(xcp[:, :, 0:DCONV - 1], 0.0)
                        for m in range(G):
                            ps = psA.tile([128, L], f32, tag="psA")
                            for fo in range(0, L, FMAX):
                                for k in range(DG):
                                    nc.tensor.matmul(
                                        out=ps[:, fo:fo + FMAX],
                                        lhsT=w_ip[k][:, m * 128:(m + 1) * 128],
                                        rhs=rr(x_ln[k][:, :])[:, fo:fo + FMAX],
                                        start=(k == 0), stop=(k == DG - 1))
                            nc.scalar.copy(out=xcp[:, m, DCONV - 1:],
                                           in_=ps[:, :])
                        w_ipz = [wpool.tile([128, DI], bf16, tag=f"wip{k}",
                                            name=f"wipz{k}") for k in range(DG)]
                        for k in range(DG):
                            nc.sync.dma_start(
                                out=w_ipz[k],
                                in_=pp["ipw"][k * 128:(k + 1) * 128, DI:])
                        for m in range(G):
                            ps = psA.tile([128, L], f32, tag="psA")
                            for fo in range(0, L, FMAX):
                                for k in range(DG):
                                    nc.tensor.matmul(
                                        out=ps[:, fo:fo + FMAX],
                                        lhsT=w_ipz[k][:, m * 128:(m + 1) * 128],
                                        rhs=rr(x_ln[k][:, :])[:, fo:fo + FMAX],
                                        start=(k == 0), stop=(k == DG - 1))
                            ztile = work.tile([128, L], bf16, tag="scr2",
                                              name="zev")
                            nc.scalar.copy(out=ztile, in_=ps[:, :])
                            nc.sync.dma_start(
                                out=z_spill[s][b, :, m * L:(m + 1) * L],
                                in_=ztile)

                        # ---- causal conv + silu -> xcv [128, GL] ----
                        xcv = big.tile([128, GL], pdt("xcv"), tag="xcv")
                        for g in range(G):
                            acc = [work.tile([128, L], f32, tag="scr4",
                                             name=f"cv{i}") for i in range(2)]
                            nc.vector.tensor_scalar_mul(
                                out=acc[0][:, :], in0=xcp[:, g, 0:L],
                                scalar1=w_cw[:, g * DCONV:g * DCONV + 1])
                            cur = 0
                            for k in range(1, DCONV):
                                nxt = 1 - cur
                                nc.vector.scalar_tensor_tensor(
                                    out=acc[nxt][:, :], in0=xcp[:, g, k:k + L],
                                    scalar=w_cw[:, g * DCONV + k:
                                                g * DCONV + k + 1],
                                    in1=acc[cur][:, :], op0=OP.mult, op1=OP.add)
                                cur = nxt
                            emit_silu(xcv[:, g * L:(g + 1) * L],
                                      acc[cur][:, :], bias=w_cb[:, g:g + 1])

                        if debug and b == 0 and l == 0 and r == 0:
                            tap("xcv00", xcv)

                        # ---- x_proj -> dtr, B, C ----
                        ps_dtr = psA.tile([128, L], f32, tag="psA")
                        ps_B = psA.tile([128, L], f32, tag="psA")
                        ps_C = psA.tile([128, L], f32, tag="psA")
                        for fo in range(0, L, FMAX):
                            for k in range(G):
                                rhs = xcv[:, k * L + fo:k * L + fo + FMAX]
                                st, sp = (k == 0), (k == G - 1)
                                nc.tensor.matmul(out=ps_dtr[0:RANK, fo:fo + FMAX],
                                                 lhsT=w_xp[k][:, 0:RANK],
                                                 rhs=rhs, start=st, stop=sp)
                                nc.tensor.matmul(out=ps_B[0:N, fo:fo + FMAX],
                                                 lhsT=w_xp[k][:, RANK:RANK + N],
                                                 rhs=rhs, start=st, stop=sp)
                                nc.tensor.matmul(out=ps_C[0:N, fo:fo + FMAX],
                                                 lhsT=w_xp[k][:, RANK + N:],
                                                 rhs=rhs, start=st, stop=sp)
                        dtr = small.tile([RANK, L], bf16, tag="dtr", bufs=2)
                        nc.scalar.copy(out=dtr, in_=ps_dtr[0:RANK, :])
                        Bsb = small.tile([N, L], pdt("BC"), tag="Bsb", bufs=2)
                        nc.scalar.copy(out=Bsb, in_=ps_B[0:N, :])
                        Csb = small.tile([N, L], pdt("BC"), tag="Csb", bufs=2)
                        nc.scalar.copy(out=Csb, in_=ps_C[0:N, :])
                        nc.sync.dma_start(out=bc_spill[s][b, 0:N, :], in_=Bsb)
                        nc.sync.dma_start(out=bc_spill[s][b, N:2 * N, :],
                                          in_=Csb)

                        # ---- dt = softplus(dtr @ dtw + dtb) ----
                        dt_sb = big.tile([128, GL], pdt("dt"), tag="dt")
                        for g in range(G):
                            ps = psA.tile([128, L], f32, tag="psA")
                            for fo in range(0, L, FMAX):
                                nc.tensor.matmul(
                                    out=ps[:, fo:fo + FMAX],
                                    lhsT=w_dt[:, g * 128:(g + 1) * 128],
                                    rhs=dtr[:, fo:fo + FMAX],
                                    start=True, stop=True)
                            sg = work.tile([128, L], f32, tag="scr4",
                                           name="sg")
                            nc.scalar.activation(
                                out=sg[:, :], in_=ps[:, :],
                                func=AF.Sigmoid, bias=w_dtb[:, g:g + 1],
                                scale=-1.0)
                            nc.scalar.activation(
                                out=dt_sb[:, g * L:(g + 1) * L], in_=sg[:, :],
                                func=AF.Ln)

                        if debug and b == 0 and l == 0 and r == 0:
                            tap("dt00", dt_sb)

                        # ---- dtx = dt * xcv ----
                        dtx = big.tile([128, GL], pdt("dtx"), tag="dtx")
                        nc.vector.scalar_tensor_tensor(
                            out=dtx[:, :], in0=dt_sb[:, :], scalar=-1.0,
                            in1=xcv[:, :], op0=OP.mult, op1=OP.mult)

                        # ---- scan over n (in two g-halves) ----
                        HG = max(1, G // 2)
                        NH = G // HG
                        HGL = HG * L
                        y_acc = big.tile([128, GL], f32, tag="xcpad",
                                         name="yacc")
                        for n in range(N):
                            brep_s = work.tile([128, L], pdt("BC"),
                                               tag="scr2", name="breps")
                            nc.sync.dma_start(
                                out=brep_s,
                                in_=bc_spill[s][b, n:n + 1, :]
                                .partition_broadcast(128))
                            crep_s = work.tile([128, L], pdt("BC"),
                                               tag="scr2", name="creps")
                            nc.sync.dma_start(
                                out=crep_s,
                                in_=bc_spill[s][b, N + n:N + n + 1, :]
                                .partition_broadcast(128))

                            for hh in range(NH):
                                sl = slice(hh * HGL, (hh + 1) * HGL)
                                dA = big.tile([128, HGL], pdt("dA"), tag="dA",
                                              name="dA")
                                nc.scalar.activation(out=dA[:, :],
                                                     in_=dt_sb[:, sl],
                                                     func=AF.Exp,
                                                     scale=float(-an[l][r][n]))
                                nc.vector.memset(
                                    dA[:, :].rearrange("p (g t) -> p g t",
                                                       g=HG)[:, :, 0:1], 0.0)
                                dxb = big.tile([128, HGL], pdt("dxb"),
                                               tag="dxb", name="dxb")
                                nc.vector.tensor_tensor(
                                    out=dxb[:, :].rearrange(
                                        "p (g t) -> p g t", g=HG),
                                    in0=dtx[:, sl].rearrange(
                                        "p (g t) -> p g t", g=HG),
                                    in1=brep_s[:, :].unsqueeze(1)
                                    .broadcast_to([128, HG, L]),
                                    op=OP.mult)
                                h_n = big.tile([128, HGL], pdt("hn"), tag="hn",
                                               name="hn")
                                nc.vector.tensor_tensor_scan(
                                    out=h_n[:, :], data0=dA[:, :],
                                    data1=dxb[:, :], initial=0.0,
                                    op0=OP.mult, op1=OP.add)
                                tmul = big.tile([128, HGL], pdt("tmul"),
                                                tag="dxb", name="tmul")
                                nc.vector.tensor_tensor(
                                    out=tmul[:, :].rearrange(
                                        "p (g t) -> p g t", g=HG),
                                    in0=h_n[:, :].rearrange(
                                        "p (g t) -> p g t", g=HG),
                                    in1=crep_s[:, :].unsqueeze(1)
                                    .broadcast_to([128, HG, L]),
                                    op=OP.mult)
                                if n == 0:
                                    nc.vector.tensor_copy(out=y_acc[:, sl],
                                                          in_=tmul[:, :])
                                else:
                                    nc.vector.tensor_tensor(
                                        out=y_acc[:, sl], in0=y_acc[:, sl],
                                        in1=tmul[:, :], op=OP.add)

                        if debug and b == 0 and l == 0 and r == 0:
                            tap("yacc00", y_acc)

                        # ---- y += D*xcv;  y *= silu(z) ----
                        for g in range(G):
                            nc.vector.scalar_tensor_tensor(
                                out=y_acc[:, g * L:(g + 1) * L],
                                in0=xcv[:, g * L:(g + 1) * L],
                                scalar=w_dv[:, g:g + 1],
                                in1=y_acc[:, g * L:(g + 1) * L],
                                op0=OP.mult, op1=OP.add)
                        z_sb = big.tile([128, GL], pdt("z"), tag="dt",
                                        name="z_sb")
                        nc.sync.dma_start(out=z_sb, in_=z_spill[s][b, :, :])
                        zs = big.tile([128, GL], pdt("z"), tag="dtx",
                                      name="zs")
                        emit_silu(zs[:, :], z_sb[:, :])
                        yg = big.tile([128, GL], pdt("yg"), tag="xcv",
                                      name="yg")
                        nc.vector.tensor_tensor(out=yg[:, :], in0=y_acc[:, :],
                                                in1=zs[:, :], op=OP.mult)

                        if debug and b == 0 and l == 0 and r == 0:
                            tap("yg00", yg)

                        # ---- out_proj (0.5 folded in); h += y_dir * mask ----
                        for m in range(DG):
                            ps = psA.tile([128, L], f32, tag="psA")
                            for fo in range(0, L, FMAX):
                                for k in range(G):
                                    nc.tensor.matmul(
                                        out=ps[:, fo:fo + FMAX],
                                        lhsT=w_op[k][:, m * 128:(m + 1) * 128],
                                        rhs=yg[:, k * L + fo:k * L + fo + FMAX],
                                        start=(k == 0), stop=(k == G - 1))
                            t1 = work.tile([128, L], f32, tag="scr4",
                                           name="cmb1")
                            nc.vector.tensor_tensor(out=t1, in0=rr(ps[:, :]),
                                                    in1=mask_rep[:, :],
                                                    op=OP.mult)
                            nc.vector.tensor_tensor(out=h_res[m][:, :],
                                                    in0=h_res[m][:, :],
                                                    in1=t1[:, :], op=OP.add)

                    if debug and b == 0:
                        tap(f"h_l{l}", h_res)

                # ======= final mask + attention pooling + proj =======
                for g in range(DG):
                    nc.vector.tensor_tensor(out=h_res[g][:, :],
                                            in0=h_res[g][:, :],
                                            in1=mask_rep[:, :], op=OP.mult)
                ps_s = psA.tile([128, L], f32, tag="psA")
                for fo in range(0, L, FMAX):
                    for g in range(DG):
                        nc.tensor.matmul(out=ps_s[0:1, fo:fo + FMAX],
                                         lhsT=sb_w2[:, g:g + 1],
                                         rhs=h_res[g][:, fo:fo + FMAX],
                                         start=(g == 0), stop=(g == DG - 1))
                # masked scores, exactly: m*s + (m*1e9 - 1e9)
                # (score_const shifts all unmasked scores uniformly; softmax
                #  is shift-invariant, so it is dropped)
                s1 = small.tile([1, L], f32, tag="row4", name="s1")
                nc.vector.tensor_tensor(out=s1, in0=ps_s[0:1, :],
                                        in1=mask_row[:, :], op=OP.mult)
                pen = small.tile([1, L], f32, tag="row4", name="pen")
                nc.vector.tensor_scalar(out=pen, in0=mask_row[:, :],
                                        scalar1=1.0e9, scalar2=1.0e9,
                                        op0=OP.mult, op1=OP.subtract)
                s3 = small.tile([1, L], f32, tag="row4", name="s3")
                nc.vector.tensor_tensor(out=s3, in0=s1[:, :], in1=pen[:, :],
                                        op=OP.add)
                rmax = small.tile([1, 1], f32, tag="row1", name="rmax")
                nc.vector.tensor_reduce(out=rmax, in_=s3[:, :],
                                        axis=mybir.AxisListType.X, op=OP.max)
                nmax = small.tile([1, 1], f32, tag="row1", name="nmax")
                nc.vector.tensor_scalar_mul(out=nmax, in0=rmax[:, :],
                                            scalar1=-1.0)
                e_s = small.tile([1, L], f32, tag="row4", name="es")
                ssum = small.tile([1, 1], f32, tag="row1", name="ssum")
                nc.scalar.activation(out=e_s, in_=s3[:, :], func=AF.Exp,
                                     bias=nmax[:, :], scale=1.0,
                                     accum_out=ssum[:, :])
                rinv = small.tile([1, 1], f32, tag="row1", name="rinv")
                nc.vector.reciprocal(out=rinv, in_=ssum[:, :])
                probs = small.tile([1, L], f32, tag="row4", name="probs")
                nc.vector.tensor_scalar_mul(out=probs, in0=e_s[:, :],
                                            scalar1=rinv[:, :])
                ps_pr = psB.tile([128, L], f32, tag="psB")
                for fo in range(0, L, FMAX):
                    nc.tensor.matmul(out=ps_pr[:, fo:fo + FMAX],
                                     lhsT=ones_row_f32[:, :],
                                     rhs=probs[:, fo:fo + FMAX],
                                     start=True, stop=True)
                rep = small.tile([128, DG], f32, tag="rep", bufs=2)
                scr = work.tile([128, L], f32, tag="scr4", name="scr")
                for g in range(DG):
                    nc.vector.scalar_tensor_tensor(
                        out=scr[:, :], in0=h_res[g][:, :], scalar=1.0,
                        in1=ps_pr[:, :], op0=OP.mult, op1=OP.mult,
                        accum_out=rep[:, g:g + 1])
                ps_o = psA.tile([128, L], f32, tag="psA")
                for g in range(DG):
                    nc.tensor.matmul(out=ps_o[0:1, 0:2],
                                     lhsT=rep[:, g:g + 1],
                                     rhs=sb_pjw[:, g * 2:(g + 1) * 2],
                                     start=(g == 0), stop=(g == DG - 1))
                fin = small.tile([1, 2], f32, tag="fin", bufs=2)
                nc.vector.tensor_tensor(out=fin, in0=ps_o[0:1, 0:2],
                                        in1=sb_pjb[:, :], op=OP.add)
                nc.sync.dma_start(out=out_t[b:b + 1, :], in_=fin)

    nc.compile()
    return nc


# --------------------------------------------------------------------------
# host side
# --------------------------------------------------------------------------

def _prep(cfg, params, embed_table, words, masks):
    B, L, DW, D, DI = cfg["B"], cfg["L"], cfg["DW"], cfg["D"], cfg["DI"]
    N, RANK, NL, NC = cfg["N"], cfg["RANK"], cfg["NL"], cfg["NCORES"]
    DG, G, DCONV = D // 128, DI // 128, cfg["DCONV"]
    BPC = B // NC

    def bf(x):
        return np.ascontiguousarray(np.asarray(x, np.float32).astype(BF16))

    def f32c(x):
        return np.ascontiguousarray(np.asarray(x, np.float32))

    def cols(vec, ng):                      # [ng*128] -> [128, ng]
        return f32c(np.asarray(vec, np.float32).reshape(ng, 128).T)

    emb = np.asarray(embed_table, np.float32)
    w = np.asarray(words).astype(np.int64)
    E = emb[w]                              # [B, L, DW]
    xT = np.ascontiguousarray(E.transpose(0, 2, 1)).astype(np.float32)

    shared = {
        "in_w": f32c(params["in_w"]),
        "in_b": cols(params["in_b"], DG),
        "lng": np.concatenate(
            [cols(lp["ln_g"], DG) for lp in params["layers"]], axis=1),
        "lnb": np.concatenate(
            [cols(lp["ln_b"], DG) for lp in params["layers"]], axis=1),
        "w2col": f32c(np.asarray(params["att_w2"], np.float32)[:, None]),
        "pjw": f32c(params["proj_w"]),
        "pjb": f32c(np.asarray(params["proj_b"], np.float32).reshape(1, 2)),
    }
    an = []
    for l, lp in enumerate(params["layers"]):
        an_l = []
        for r, key in enumerate(("fwd", "bwd")):
            mp = lp[key]
            A = -np.exp(np.asarray(mp["A_log"], np.float32))
            assert np.allclose(A, A[0:1, :], rtol=1e-5), "A not d-independent"
            an_l.append([float(v) for v in A[0]])
            s = f"{l}{r}"
            shared[f"ipw_{s}"] = bf(mp["in_proj"])
            cwv = np.asarray(mp["conv_w"], np.float32)      # [DI, DCONV]
            shared[f"cw_{s}"] = f32c(
                cwv.reshape(G, 128, DCONV).transpose(1, 0, 2).reshape(128, -1))
            shared[f"cb_{s}"] = cols(mp["conv_b"], G)
            shared[f"xpw_{s}"] = bf(mp["x_proj"])
            shared[f"dtw_{s}"] = bf(mp["dt_w"])
            shared[f"dtb_{s}"] = cols(-np.asarray(mp["dt_b"], np.float32), G)
            shared[f"dv_{s}"] = cols(mp["D"], G)
            shared[f"opw_{s}"] = bf(0.5 * np.asarray(mp["out_proj"], np.float32))
        an.append(an_l)

    sc = float(np.dot(np.asarray(params["att_guide"], np.float32),
                      np.asarray(params["att_w1"], np.float32))
               + np.asarray(params["att_b"], np.float32))

    in_maps = []
    for c in range(NC):
        m = dict(shared)
        m["xT"] = np.ascontiguousarray(xT[c * BPC:(c + 1) * BPC])
        m["mask"] = bf(np.asarray(masks, np.float32)[c * BPC:(c + 1) * BPC])
        in_maps.append(m)
    return in_maps, an, sc


def kernel(params, embed_table, words, masks, word_len):
    cfg = dict(FULL)
    cfg["BPC"] = cfg["B"] // cfg["NCORES"]
    in_maps, an, sc = _prep(cfg, params, embed_table, words, masks)
    if "full" not in _nc_cache:
        bcfg = dict(cfg)
        bcfg["an"], bcfg["score_const"] = an, sc
        _nc_cache["full"] = build_nc(bcfg)
    nc = _nc_cache["full"]
    res = run_bass_kernel_spmd(nc, in_maps, core_ids=list(range(cfg["NCORES"])))
    out = np.concatenate([r["out"] for r in res.results], axis=0)
    return out.astype(np.float32)
